# revision 13
# baseline (speedup 1.0000x reference)
"""Trainium2 Bass kernel for nn_PatchMMConvolution.

Computes a shared-weight 3x3 conv (stride 1, pad 1) over x[B=2, P=18, Cin=64,
H=128, W=128] with weight[Cout=128, Cin=64, 3, 3] + bias, i.e. conv2d on
36 images, returning [2, 18, 128, 128, 128] float32.

Strategy (8 NeuronCores, SPMD single program):
  - 36 images are split into 16 "streams" of 288 output rows each
    (2 full images + one quarter-image per stream). Each core runs two
    streams: stream A in SBUF partitions 0-63, stream B in partitions 64-127
    (Cin=64 channels live on partitions).
  - Host pre-pads each stream into a fp16 "slab" [64, 294, 130]: three
    vertically concatenated zero-padded segments (130+130+34 rows, W -> 130).
  - Conv is 9 shifted matmuls accumulating in PSUM. All wire data is fp16
    (inputs, weights, outputs); accumulation is fp32 in PSUM. fp16 halves
    HBM traffic and enables fast weight load (FWL) on the PE.
  - Loop: per 32-output-row chunk, 4 "superchunks" of 8 rows; within a
    superchunk the tap loop is OUTER and each tap issues 2 matmuls per
    stream (weight reuse), streams A/B on PE row groups 0-1/2-3 run
    concurrently. PSUM: 4 tags x 2 bufs = 8 banks (double buffered).
  - Eviction: Vector engine tensor_scalar_add(psum + bias) -> fp16 staging
    tile [128, 32, 128]; one 1MB DMA per stream per chunk to DRAM.
  - Host upcasts the fp16 output to fp32.
"""

import numpy as np

import concourse.bass as bass
import concourse.mybir as mybir
import concourse.tile as tile
from concourse import bacc
from concourse._compat import get_trn_type
from concourse.bass_utils import run_bass_kernel_spmd

B, PP, CIN, H, W = 2, 18, 64, 128, 128
COUT = 128
NIMG = B * PP  # 36
NCORES = 8
NSTREAM = 16
WP = W + 2  # 130 padded width
RSLAB = 294  # 130 + 130 + 34 slab rows per stream
ROWS_PER_STREAM = 288
# (slab_row_base, out_row_base, out_rows) per segment
SEGS = [(0, 0, 128), (130, 128, 128), (260, 256, 32)]
CHUNK_OUT_ROWS = 32  # output rows per input chunk
CHUNK_ROWS = CHUNK_OUT_ROWS + 2  # 34 input rows per chunk
TILE_OUT_ROWS = 4  # output rows per matmul tile (4*128 = 512 = one PSUM bank)
SC_TILES = 2  # matmul tiles per stream per superchunk (weight reuse factor)
SC_OUT_ROWS = SC_TILES * TILE_OUT_ROWS  # 8 output rows per superchunk

DT = mybir.dt.float16  # wire dtype for x and weights
ODT = mybir.dt.float16  # wire dtype for output
ACC = mybir.dt.float32

# Benchmark knob: repeat the whole kernel body KERNEL_REPS times inside a
# hardware loop (used to isolate device exec time from dispatch overhead).
KERNEL_REPS = 1
IN_BUFS = 3  # input chunk buffering depth
STG_BUFS = 2  # output staging buffering depth
# Ablation knobs (timing probes only; break correctness when True)
NO_EVICT = False  # skip PSUM->SBUF eviction (and bias)
NO_MM = False  # skip the matmuls
# Split each matmul into COL_SPLIT column-group matmuls of M=128/COL_SPLIT.
# Smaller col regions shrink the per-region LDWEIGHTS exposure (P cols/1.2GHz)
# while the regions run concurrently on the PE. Measured: 2 is best (188us vs
# 197us at 1, 217us at 4 -- the 4-way split doubles instruction/semaphore
# overhead on the PE queue).
COL_SPLIT = 2
# Evict half the PSUM banks on the Scalar (ACT) engine instead of Vector:
# activation(Identity, bias=b, scale=1) does the same psum+bias -> fp16 copy.
ACT_EVICT = False

_PROGRAM = None


def _build_program():
    nc = bacc.Bacc(get_trn_type() or "TRN2", target_bir_lowering=False)
    xs = nc.dram_tensor("xs", [128, RSLAB, WP], DT, kind="ExternalInput")
    wd = nc.dram_tensor("wt", [128, 9, COUT], DT, kind="ExternalInput")
    bd = nc.dram_tensor("bias", [COUT, 1], ACC, kind="ExternalInput")
    od = nc.dram_tensor(
        "out", [COUT, 2, ROWS_PER_STREAM, W], ODT, kind="ExternalOutput"
    )

    chunks = []
    for sb, ob, nr in SEGS:
        for j in range(nr // CHUNK_OUT_ROWS):
            chunks.append((sb + CHUNK_OUT_ROWS * j, ob + CHUNK_OUT_ROWS * j))

    psum_bufs = max(1, 8 // (2 * SC_TILES))
    with tile.TileContext(nc) as tc:
        with (
            tc.tile_pool(name="const", bufs=1) as cpool,
            tc.tile_pool(name="inp", bufs=IN_BUFS) as ipool,
            tc.tile_pool(name="stg", bufs=STG_BUFS) as spool,
            tc.tile_pool(name="ps", bufs=psum_bufs, space="PSUM") as pspool,
        ):
            w_sb = cpool.tile([128, 9, COUT], DT)
            nc.sync.dma_start(w_sb[:], wd[:])
            b_sb = cpool.tile([COUT, 1], ACC)
            nc.sync.dma_start(b_sb[:], bd[:])

            def emit_body():
                for srow, orow in chunks:
                    ch = ipool.tile([128, CHUNK_ROWS, WP], DT, tag="chunk")
                    nc.sync.dma_start(ch[:], xs[:, srow : srow + CHUNK_ROWS, :])
                    stA = spool.tile([128, CHUNK_OUT_ROWS, W], ODT, tag="stA")
                    stB = spool.tile([128, CHUNK_OUT_ROWS, W], ODT, tag="stB")
                    sc_out_rows = SC_TILES * TILE_OUT_ROWS
                    for k in range(CHUNK_OUT_ROWS // sc_out_rows):
                        r0 = sc_out_rows * k
                        ps = [
                            [
                                pspool.tile(
                                    [128, TILE_OUT_ROWS, W],
                                    ACC,
                                    tag=f"ps{s}{j}",
                                    name=f"ps{s}{j}",
                                )
                                for j in range(SC_TILES)
                            ]
                            for s in ("A", "B")
                        ]
                        if not NO_MM:
                            mw = 128 // COL_SPLIT
                            for tap in range(9):
                                kh, kw = divmod(tap, 3)
                                first, last = tap == 0, tap == 8
                                for s in range(2):
                                    for j in range(SC_TILES):
                                        rr = r0 + TILE_OUT_ROWS * j + kh
                                        rhs = ch[
                                            64 * s : 64 * s + 64,
                                            rr : rr + TILE_OUT_ROWS,
                                            kw : kw + W,
                                        ]
                                        for c in range(COL_SPLIT):
                                            wv = w_sb[
                                                64 * s : 64 * s + 64,
                                                tap,
                                                c * mw : (c + 1) * mw,
                                            ]
                                            nc.tensor.matmul(
                                                ps[s][j][c * mw : (c + 1) * mw],
                                                wv,
                                                rhs,
                                                start=first,
                                                stop=last,
                                                tile_position=(64 * s, c * mw),
                                            )
                        if not NO_EVICT:
                            for s, stg in ((0, stA), (1, stB)):
                                for j in range(SC_TILES):
                                    rr = r0 + TILE_OUT_ROWS * j
                                    dst = stg[:, rr : rr + TILE_OUT_ROWS, :]
                                    if ACT_EVICT and j % 2 == 1:
                                        nc.scalar.activation(
                                            dst,
                                            ps[s][j][:],
                                            mybir.ActivationFunctionType.Identity,
                                            bias=b_sb[:],
                                            scale=1.0,
                                        )
                                    else:
                                        nc.vector.tensor_scalar_add(
                                            dst, ps[s][j][:], b_sb[:]
                                        )
                    nc.sync.dma_start(
                        od[:, 0, orow : orow + CHUNK_OUT_ROWS, :], stA[:]
                    )
                    nc.sync.dma_start(
                        od[:, 1, orow : orow + CHUNK_OUT_ROWS, :], stB[:]
                    )

            if KERNEL_REPS > 1:
                with tc.For_i(0, KERNEL_REPS, 1) as _i:
                    emit_body()
            else:
                emit_body()
    nc.finalize()
    return nc


def _get_program():
    global _PROGRAM
    if _PROGRAM is None:
        _PROGRAM = _build_program()
    return _PROGRAM


def _stream_parts(s):
    """Stream s covers full images 2s, 2s+1 and quarter (s%4) of image 32+(s//4)...
    returns (img0, img1, img_q, q) with quarter rows [32q, 32q+32)."""
    img_q = 32 + (s % 4)
    q = s // 4
    return 2 * s, 2 * s + 1, img_q, q


def _make_slab(X, s):
    """Build padded slab [CIN, RSLAB, WP] for stream s from X [NIMG,CIN,H,W]."""
    i0, i1, iq, q = _stream_parts(s)
    sl = np.zeros((CIN, RSLAB, WP), np.float16)
    sl[:, 1 : H + 1, 1 : W + 1] = X[i0]
    sl[:, 131 : 131 + H, 1 : W + 1] = X[i1]
    r0 = 32 * q
    lo, hi = max(r0 - 1, 0), min(r0 + 33, H)
    d0 = 260 + (lo - (r0 - 1))
    sl[:, d0 : d0 + (hi - lo), 1 : W + 1] = X[iq, :, lo:hi]
    return sl


def make_in_maps(x, weight, bias):
    x = np.asarray(x, dtype=np.float32)
    weight = np.asarray(weight, dtype=np.float32)
    bias = np.ascontiguousarray(np.asarray(bias), dtype=np.float32)
    X = x.reshape(NIMG, CIN, H, W).astype(np.float16)

    wt = np.ascontiguousarray(
        weight.transpose(1, 2, 3, 0).reshape(CIN, 9, COUT)
    ).astype(np.float16)
    wt2 = np.ascontiguousarray(np.concatenate([wt, wt], axis=0))  # [128, 9, COUT]
    bb = np.ascontiguousarray(bias.reshape(COUT, 1))

    in_maps = []
    for c in range(NCORES):
        xs = np.concatenate([_make_slab(X, 2 * c), _make_slab(X, 2 * c + 1)], axis=0)
        in_maps.append({"xs": np.ascontiguousarray(xs), "wt": wt2, "bias": bb})
    return in_maps


def kernel(x, weight, bias):
    in_maps = make_in_maps(x, weight, bias)
    nc = _get_program()
    res = run_bass_kernel_spmd(nc, in_maps, core_ids=list(range(NCORES)))

    Y = np.empty((NIMG, COUT, H, W), np.float32)
    for c in range(NCORES):
        o = res.results[c]["out"]  # [COUT, 2, 288, W] fp16
        for half in (0, 1):
            s = 2 * c + half
            i0, i1, iq, q = _stream_parts(s)
            oo = o[:, half].astype(np.float32)
            Y[i0] = oo[:, 0:H]
            Y[i1] = oo[:, H : 2 * H]
            Y[iq, :, 32 * q : 32 * q + 32, :] = oo[:, 2 * H : 2 * H + 32]
    return Y.reshape(B, PP, COUT, H, W)
